# revision 36
# baseline (speedup 1.0000x reference)
"""MaskEnhancer kernel for 8 Trainium2 NeuronCores.

Math (from the reference): out0 = feat0 and out1 = feat1 are identity
passthroughs (the attention computed for the first two layers is
discarded).  Only the last layer does work:

    sam_sub = sam_masks[:, 1:, ::16, ::16]           # nearest-resize to 32x32
    win(p)  = 1 + argmax-style last mask covering pixel p (0 = background)
    tm[m,p] = onehot                                  # [32, 1024] per batch
    mf      = l2norm( (tm @ feat2_flat^T) / (cnt + 1e-5) )
    q       = mf @ W2.T + b2
    cos     = (q q^T) / (|q_i||q_j| + 1e-8)
    gp      = l2norm( diag(1/(rowsum(cos)+1e-8)) cos @ q )
    mf_f    = mf + sigmoid(g2) * gp
    out2    = feat2 + reshape(mf_f^T @ tm)

Sharding: 8 cores = 4 batches x 2 channel-halves.  Each core reads the
full (channel-rolled) feature map of its batch to compute the full
pooled/attended mask features redundantly (no collectives), then emits
out2 for its 512-channel half.  The channel roll makes the SPMD program
identical across cores: every core's own channels are [0:512] in its
rolled coordinate system.

The pooling contraction needs feat^T ([pixel, channel]); that layout is
produced with the DMA x-bar transpose, which only supports 2-byte
dtypes, so a bf16 copy of feat2 is staged host-side.  bf16 is only used
for matmul operands whose error is attenuated (pool sums, W projection);
the residual add out2 = feat2 + fs stays fp32.  Validated end-to-end
rel. error ~1e-4.
"""

import os
import tempfile

import numpy as np


def _install_trace_hook():
    """Bridge trn_agent_boot's NTFF profiling into antenv.axon_hooks
    (dev-only path, used when KERNEL_TRACE=1)."""
    import sys, types
    try:
        import antenv.axon_hooks  # noqa: F401
        return
    except ImportError:
        pass
    from trn_agent_boot.trn_boot import _ntff_profile_via_ctypes
    hook = _ntff_profile_via_ctypes("/opt/axon/libaxon_pjrt.so")
    mod = types.ModuleType("antenv.axon_hooks")
    mod.get_axon_ntff_profile_hook = lambda: hook
    mod.set_axon_ntff_profile_hook = lambda h: None
    sys.modules["antenv.axon_hooks"] = mod
    import antenv
    antenv.axon_hooks = mod
    from concourse import bass_utils
    bass_utils.upload_artifacts = lambda tmpdir: f"file://{tmpdir}"


B = 4
M = 32        # masks after dropping the first
C = 1024      # feat2 channels
P = 1024      # 32*32 pixels
CH = 512      # channels per core
N_CORES = 8

_CACHE = {}


def _build_program():
    import concourse.bass as bass
    import concourse.tile as tile
    from concourse import mybir, bacc
    from concourse.masks import make_identity

    f32 = mybir.dt.float32
    bf16 = mybir.dt.bfloat16
    i32 = mybir.dt.int32
    Alu = mybir.AluOpType
    Act = mybir.ActivationFunctionType

    nc = bacc.Bacc("TRN2", target_bir_lowering=False, enable_partition_id=False)

    featTd = nc.dram_tensor("featT", [P, C], bf16, kind="ExternalInput")
    feath = nc.dram_tensor("feath", [CH, P], f32, kind="ExternalInput")
    wt = nc.dram_tensor("wt", [C, C], bf16, kind="ExternalInput")
    sam = nc.dram_tensor("sam", [P, M], i32, kind="ExternalInput")
    bias = nc.dram_tensor("bias", [1, C], f32, kind="ExternalInput")
    sig = nc.dram_tensor("sig", [1], f32, kind="ExternalInput")
    out = nc.dram_tensor("out", [CH, P], f32, kind="ExternalOutput")

    with tile.TileContext(nc) as tc:
      with tc.tile_pool(name="persist", bufs=1) as persist:
        def _t(tc, shape, dtype, name):
            return persist.tile(shape, dtype, tag=name, name=name)

        # ---- constants ------------------------------------------------
        ident_f = _t(tc, [128, 128], f32, "ident_f")
        make_identity(nc, ident_f)
        ident_b = _t(tc, [128, 128], bf16, "ident_b")
        make_identity(nc, ident_b)
        ones_b = _t(tc, [128, 1], bf16, "ones_b")
        nc.vector.memset(ones_b[:], 1.0)
        iota1 = _t(tc, [128, 8, M], i32, "iota1")
        nc.gpsimd.iota(iota1[:], pattern=[[0, 8], [1, M]], base=1,
                       channel_multiplier=0)

        bias_bc = _t(tc, [M, C], f32, "bias_bc")
        bias_ap = bias[:]
        nc.gpsimd.dma_start(
            out=bias_bc[:],
            in_=bass.AP(tensor=bias_ap.tensor, offset=bias_ap.offset,
                        ap=[[0, M]] + list(bias_ap.ap[1:])))
        sig_t = _t(tc, [M, 1], f32, "sig_t")
        sig_ap = sig[:]
        nc.gpsimd.dma_start(
            out=sig_t[:],
            in_=bass.AP(tensor=sig_ap.tensor, offset=sig_ap.offset,
                        ap=[[0, M]] + list(sig_ap.ap)))

        # ---- big loads ------------------------------------------------
        # feat^T is staged host-side (bf16, [pixel, channel]) so no DMA
        # x-bar transposes are needed; loads spread over HWDGE + SWDGE.
        sam_all = _t(tc, [128, 8, M], i32, "sam_all")
        sam_r = sam.rearrange("(j p) m -> p j m", p=128)
        nc.sync.dma_start(out=sam_all[:], in_=sam_r)

        # Loads are phase-sequenced (featT -> wt -> feath) with explicit
        # deps so the earlier-needed tensors get the full HBM bandwidth
        # instead of fair-sharing with loads only needed much later.
        featT = _t(tc, [128, 8, C], bf16, "featT")   # [pix, ptile, ch]
        featT_r = featTd.rearrange("(j p) c -> p j c", p=128)
        ft_dmas = []
        for j in range(8):
            eng = nc.sync if j % 2 == 0 else nc.scalar
            ft_dmas.append(
                eng.dma_start(out=featT[:, j, :], in_=featT_r[:, j, :]))

        wt_all = _t(tc, [128, 8, C], bf16, "wt_all")  # [cin, ktile, cout]
        wt_dmas = []
        for k in range(8):
            eng = nc.sync if k % 2 == 0 else nc.scalar
            wt_dmas.append(
                eng.dma_start(out=wt_all[:, k, :],
                              in_=wt[k * 128:(k + 1) * 128, :]))

        # feath is only needed by the final residual add; gate it behind
        # wt (and keep it on sync, which runs no compute) so featT/wt get
        # the HBM bandwidth first.
        feath_all = _t(tc, [128, 4, P], f32, "feath_all")
        for i in range(4):
            fh = nc.sync.dma_start(out=feath_all[:, i, :],
                                   in_=feath[i * 128:(i + 1) * 128, :])
            for dep in wt_dmas[6:]:
                tile.add_dep_helper(fh.ins, dep.ins, reason="feath after wt")

        with (
            tc.tile_pool(name="work", bufs=2) as work,
            tc.tile_pool(name="opool", bufs=4) as opool,
            tc.tile_pool(name="psA", bufs=1, space=bass.MemorySpace.PSUM) as psA,
            tc.tile_pool(name="psQ", bufs=1, space=bass.MemorySpace.PSUM) as psQ,
            tc.tile_pool(name="psT", bufs=2, space=bass.MemorySpace.PSUM) as psT,
            tc.tile_pool(name="psF", bufs=2, space=bass.MemorySpace.PSUM) as psF,
        ):
            # PSUM budget (8 banks): psA tag "acc" = 2 banks (ring of 1:
            # mf -> q -> simq), psT tag "sm" = 2 banks (ring of 2, all
            # small tiles), psF tag "fs" = 4 banks (ring of 2, shared by
            # the tm transposes and the fs accumulators).
            ones_col = _t(tc, [M, 1], bf16, "ones_col")
            nc.vector.memset(ones_col[:], 1.0)

            # preload the ACT function table used by Sqrt/Square off the
            # critical path
            one_f = work.tile([1, 1], f32, tag="one_f")
            nc.vector.memset(one_f[:], 1.0)
            warm_act = work.tile([1, 1], f32, tag="warm_act")
            nc.scalar.sqrt(warm_act[:], one_f[:])

            # Ungated junk matmuls dropped into the PE stream at phase
            # boundaries: they fill PE-idle windows (keeping the HAM
            # clock-gate at 2.4 GHz) and never gate real work.
            def _warm(tag, n=4, rhs=None, k=128):
                if rhs is None:
                    rhs = featT[:, 0, 0:512]
                wps = psF.tile([128, 512], f32, tag="fs", name=f"w_{tag}")
                nfree = rhs.free_size()
                for w in range(n):
                    nc.tensor.matmul(wps[:, 0:nfree], lhsT=ident_b[0:k, :],
                                     rhs=rhs, start=(w == 0), stop=(w == n - 1))
                sink = work.tile([1, 1], f32, tag="sink", name=f"s_{tag}")
                nc.vector.tensor_copy(sink[:], wps[0:1, 0:1])

            # ---- one-hot masks (tm) ----------------------------------
            t1 = work.tile([128, 8, M], i32, tag="t1")
            nc.vector.tensor_tensor(out=t1[:], in0=sam_all[:], in1=iota1[:],
                                    op=Alu.mult)
            win = work.tile([128, 8], i32, tag="win")
            nc.vector.reduce_max(out=win[:], in_=t1[:],
                                 axis=mybir.AxisListType.X)
            win_f = work.tile([128, 8], f32, tag="win_f")
            nc.vector.tensor_copy(win_f[:], win[:])
            iota_f = _t(tc, [128, 8, M], f32, "iota_f")
            nc.vector.tensor_copy(iota_f[:], iota1[:])
            tm_all = _t(tc, [128, 8, M], bf16, "tm_all")
            for j in range(8):
                nc.vector.tensor_scalar(
                    out=tm_all[:, j, :], in0=iota_f[:, j, :],
                    scalar1=win_f[:, j:j + 1], scalar2=None, op0=Alu.is_equal)

            # ---- tm^T ([mask, pixel]) + counts, early ----------------
            tm_mp = _t(tc, [M, P], bf16, "tm_mp")
            tt_ps = psT.tile([M, P], bf16, tag="sm")
            for j in range(8):
                nc.tensor.transpose(tt_ps[:, j * 128:(j + 1) * 128],
                                    tm_all[:, j, :], ident_b[:])
            nc.vector.tensor_copy(tm_mp[:], tt_ps[:])
            cnt_s = work.tile([M, 1], f32, tag="cnt_s")
            nc.vector.reduce_sum(out=cnt_s[:], in_=tm_mp[:],
                                 axis=mybir.AxisListType.X)

            _warm("w0")

            # ---- masked pooling: mf = tm^T @ featT -------------------
            mf_ps = psA.tile([M, C], f32, tag="acc")
            for j in range(8):
                st, sp = (j == 0), (j == 7)
                nc.tensor.matmul(mf_ps[:, 0:512], lhsT=tm_all[:, j, :],
                                 rhs=featT[:, j, 0:512], start=st, stop=sp)
                nc.tensor.matmul(mf_ps[:, 512:1024], lhsT=tm_all[:, j, :],
                                 rhs=featT[:, j, 512:1024], start=st, stop=sp)

            # ---- raw-mf^T (bf16): emitted first so the PSUM-reader
            # serialization doesn't chain the q path behind the norm chain
            mf_sb = _t(tc, [M, C], bf16, "mf_sb")
            nc.scalar.copy(mf_sb[:], mf_ps[:])
            mfnT = _t(tc, [128, 8, M], bf16, "mfnT")
            tp_ps = psT.tile([128, 8, M], bf16, tag="sm")
            for j in range(8):
                nc.tensor.transpose(tp_ps[:, j, :],
                                    mf_sb[:, j * 128:(j + 1) * 128],
                                    ident_b[0:M, 0:M])
            nc.scalar.copy(mfnT[:], tp_ps[:])

            # ---- normalize: mf_n = l2norm(mf / (cnt + 1e-5)) ---------
            nc.vector.tensor_scalar_add(cnt_s[:], cnt_s[:], 1e-5)
            inv_c = work.tile([M, 1], f32, tag="inv_c")
            nc.vector.reciprocal(inv_c[:], cnt_s[:])
            sq_scr = _t(tc, [M, C], f32, "sq_scr")
            n2p = work.tile([M, 1], f32, tag="n2p")
            nc.scalar.activation(sq_scr[:], mf_ps[:], Act.Square,
                                 accum_out=n2p[:])
            npool = work.tile([M, 1], f32, tag="npool")
            nc.scalar.sqrt(npool[:], n2p[:])
            den = work.tile([M, 1], f32, tag="den")
            nc.vector.tensor_scalar(out=den[:], in0=npool[:],
                                    scalar1=inv_c[:, 0:1], scalar2=1e-12,
                                    op0=Alu.mult, op1=Alu.max)
            invd = work.tile([M, 1], f32, tag="invd")
            nc.vector.reciprocal(invd[:], den[:])
            scale_mf = work.tile([M, 1], f32, tag="scale_mf")
            nc.vector.tensor_mul(scale_mf[:], inv_c[:], invd[:])
            mf_n = _t(tc, [M, C], f32, "mf_n")
            nc.vector.tensor_scalar_mul(mf_n[:], mf_ps[:], scale_mf[:, 0:1])

            _warm("wa")

            # ---- q_raw = mf @ W^T; q = scale*q_raw + b afterwards ----
            # (the l2norm row-scale commutes past W^T, so the projection
            # overlaps the norm chain instead of waiting for it)
            q_ps = psQ.tile([M, C], f32, tag="q")
            for k in range(8):
                st, sp = (k == 0), (k == 7)
                nc.tensor.matmul(q_ps[:, 0:512], lhsT=mfnT[:, k, :],
                                 rhs=wt_all[:, k, 0:512], start=st, stop=sp)
                nc.tensor.matmul(q_ps[:, 512:1024], lhsT=mfnT[:, k, :],
                                 rhs=wt_all[:, k, 512:1024], start=st,
                                 stop=sp)

            _warm("wb")

            q_b = _t(tc, [M, C], bf16, "q_b")
            nc.vector.scalar_tensor_tensor(
                out=q_b[:], in0=q_ps[:], scalar=scale_mf[:, 0:1],
                in1=bias_bc[:], op0=Alu.mult, op1=Alu.add)
            sq_scr2 = _t(tc, [M, C], f32, "sq_scr2")
            n2q = work.tile([M, 1], f32, tag="n2q")
            nc.scalar.activation(sq_scr2[:], q_b[:], Act.Square,
                                 accum_out=n2q[:])
            n_q = work.tile([M, 1], f32, tag="n_q")
            nc.scalar.sqrt(n_q[:], n2q[:])

            # ---- cos = (q q^T) / (outer(n, n) + 1e-8) ----------------
            nps = psT.tile([1, M], f32, tag="sm")
            nc.tensor.transpose(nps[:], n_q[:], ident_f[0:M, 0:M])
            n_row = work.tile([1, M], f32, tag="n_row")
            nc.scalar.copy(n_row[:], nps[:])
            outer_ps = psT.tile([M, M], f32, tag="sm")
            nc.tensor.matmul(outer_ps[:], lhsT=n_row[:], rhs=n_row[:],
                             start=True, stop=True)
            den2 = work.tile([M, M], f32, tag="den2")
            nc.vector.tensor_scalar_add(den2[:], outer_ps[:], 1e-8)
            nc.vector.reciprocal(den2[:], den2[:])

            qT = _t(tc, [128, 8, M], bf16, "qT")
            tq_ps = psT.tile([128, 8, M], bf16, tag="sm")
            for j in range(8):
                nc.tensor.transpose(tq_ps[:, j, :],
                                    q_b[:, j * 128:(j + 1) * 128],
                                    ident_b[0:M, 0:M])
            nc.vector.tensor_copy(qT[:], tq_ps[:])

            dot_ps = psT.tile([M, M], f32, tag="sm")
            for j in range(8):
                nc.tensor.matmul(dot_ps[:], lhsT=qT[:, j, :], rhs=qT[:, j, :],
                                 start=(j == 0), stop=(j == 7))
            _warm("wc", rhs=q_b[:, 0:512], k=M)
            cos_b = work.tile([M, M], bf16, tag="cos_b")
            nc.vector.tensor_mul(cos_b[:], dot_ps[:], den2[:])

            # ---- simq = cos @ q ; rowsum = cos @ 1 -------------------
            sq_ps = psQ.tile([M, C], f32, tag="q")
            for s, e in ((0, 512), (512, 1024)):
                nc.tensor.matmul(sq_ps[:, s:e], lhsT=cos_b[:], rhs=q_b[:, s:e],
                                 start=True, stop=True)
            rs_ps = psT.tile([M, 1], f32, tag="sm")
            nc.tensor.matmul(rs_ps[:], lhsT=cos_b[:], rhs=ones_col[:],
                             start=True, stop=True)
            _warm("wd", rhs=qT[:, 0:8, :], k=128)

            # mf_f = mf_n + coef * simq, with
            # coef = sig * rs / max(|rs| * ||simq_row||, 1e-12),
            # rs = 1/(rowsum + 1e-8)   (== sig * l2norm of the gp row)
            rs = work.tile([M, 1], f32, tag="rs")
            nc.vector.tensor_scalar_add(rs[:], rs_ps[:], 1e-8)
            nc.vector.reciprocal(rs[:], rs[:])
            sq_scr3 = _t(tc, [M, C], f32, "sq_scr3")
            n2g = work.tile([M, 1], f32, tag="n2g")
            nc.scalar.activation(sq_scr3[:], sq_ps[:], Act.Square,
                                 accum_out=n2g[:])
            nsq = work.tile([M, 1], f32, tag="nsq")
            nc.scalar.sqrt(nsq[:], n2g[:])
            rneg = work.tile([M, 1], f32, tag="rneg")
            nc.vector.tensor_scalar_mul(rneg[:], rs[:], -1.0)
            rabs = work.tile([M, 1], f32, tag="rabs")
            nc.vector.tensor_max(rabs[:], rs[:], rneg[:])
            den3 = work.tile([M, 1], f32, tag="den3")
            nc.vector.tensor_mul(den3[:], rabs[:], nsq[:])
            nc.vector.tensor_scalar_max(den3[:], den3[:], 1e-12)
            inv3 = work.tile([M, 1], f32, tag="inv3")
            nc.vector.reciprocal(inv3[:], den3[:])
            coef = work.tile([M, 1], f32, tag="coef")
            nc.vector.tensor_mul(coef[:], rs[:], inv3[:])
            nc.vector.tensor_mul(coef[:], coef[:], sig_t[:])

            mf_fb = _t(tc, [M, CH], bf16, "mf_fb")
            nc.vector.scalar_tensor_tensor(
                out=mf_fb[:], in0=sq_ps[:, 0:CH], scalar=coef[:, 0:1],
                in1=mf_n[:, 0:CH], op0=Alu.mult, op1=Alu.add)

            # ---- fs = mf_f^T @ tm ; out = feat + fs ------------------
            # The residual adds are spread over DVE (direct from PSUM)
            # and ACT-copy + GpSimd-add so no single engine serializes
            # the tail; stores split across the two HWDGE queues.
            for i in range(4):
                for hx, (s, e) in enumerate(((0, 512), (512, 1024))):
                    u = i * 2 + hx
                    fs_ps = psF.tile([128, 512], f32, tag="fs")
                    nc.tensor.matmul(fs_ps[:],
                                     lhsT=mf_fb[:, i * 128:(i + 1) * 128],
                                     rhs=tm_mp[:, s:e], start=True, stop=True)
                    o_t = opool.tile([128, 512], f32, tag="o_t")
                    if u % 2 == 0:
                        nc.vector.tensor_add(o_t[:], fs_ps[:],
                                             feath_all[:, i, s:e])
                    else:
                        fs_sb = opool.tile([128, 512], f32, tag="fs_sb")
                        nc.scalar.copy(fs_sb[:], fs_ps[:])
                        nc.gpsimd.tensor_tensor(out=o_t[:], in0=fs_sb[:],
                                                in1=feath_all[:, i, s:e],
                                                op=Alu.add)
                    eng = nc.sync if u % 2 == 0 else nc.scalar
                    eng.dma_start(out=out[i * 128:(i + 1) * 128, s:e],
                                  in_=o_t[:])

    nc.compile()
    return nc


def _get_program():
    if "nc" not in _CACHE:
        _CACHE["nc"] = _build_program()
    return _CACHE["nc"]


def kernel(feat0, feat1, feat2, sam_masks, W0, b0, W1, b1, W2, b2, g0, g1, g2):
    import ml_dtypes
    from concourse.bass_utils import run_bass_kernel_spmd

    bf16 = ml_dtypes.bfloat16
    feat0 = np.asarray(feat0, np.float32)
    feat1 = np.asarray(feat1, np.float32)
    feat2 = np.asarray(feat2, np.float32)
    sam_masks = np.asarray(sam_masks, np.int32)
    W2 = np.asarray(W2, np.float32)
    b2 = np.asarray(b2, np.float32)
    g2 = np.float32(np.asarray(g2))

    nc = _get_program()

    feat2_r = feat2.reshape(B, C, P)
    fT = np.ascontiguousarray(feat2_r.transpose(0, 2, 1)).astype(bf16)
    fT_roll = np.ascontiguousarray(
        np.concatenate([fT[:, :, CH:], fT[:, :, :CH]], axis=2))  # h=1 variant

    WT = np.ascontiguousarray(W2.T).astype(bf16)  # [cin, cout]
    WT_roll = np.ascontiguousarray(
        np.roll(np.roll(WT, -CH, axis=0), -CH, axis=1))
    b_roll = np.roll(b2, -CH)
    b2_bf = np.ascontiguousarray(b2.reshape(1, C))
    b_roll_bf = np.ascontiguousarray(b_roll.reshape(1, C))
    sig = np.float32(1.0) / (np.float32(1.0) + np.exp(-g2, dtype=np.float32))
    sig = np.asarray([sig], np.float32)

    # [B, p, m] int32, pixels at stride 16 of the 512-grid
    sam_pm = np.ascontiguousarray(
        sam_masks[:, 1:, ::16, ::16].reshape(B, M, P).transpose(0, 2, 1))

    in_maps = []
    for core in range(N_CORES):
        b, h = divmod(core, 2)
        in_maps.append({
            "featT": fT[b] if h == 0 else fT_roll[b],
            "feath": np.ascontiguousarray(feat2_r[b, h * CH:(h + 1) * CH]),
            "wt": WT if h == 0 else WT_roll,
            "sam": sam_pm[b],
            "bias": (b2_bf if h == 0 else b_roll_bf),
            "sig": sig,
        })

    trace = bool(int(os.environ.get("KERNEL_TRACE", "0")))
    kw = {}
    if trace:
        _install_trace_hook()
        kw = dict(trace=True, tmpdir=tempfile.mkdtemp(prefix="kernel_trace_"))
        tc_env = os.environ.get("KERNEL_TRACE_CORES")
        if tc_env:
            kw["trace_cores"] = [int(x) for x in tc_env.split(",")]
    res = run_bass_kernel_spmd(nc, in_maps, core_ids=list(range(N_CORES)), **kw)
    _CACHE["last_result"] = res

    out2 = np.empty((B, C, P), np.float32)
    for core in range(N_CORES):
        b, h = divmod(core, 2)
        out2[b, h * CH:(h + 1) * CH] = res.results[core]["out"]

    return feat0, feat1, out2.reshape(B, C, 32, 32)


# revision 37
# speedup vs baseline: 1.0233x; 1.0233x over previous
"""MaskEnhancer kernel for 8 Trainium2 NeuronCores.

Math (from the reference): out0 = feat0 and out1 = feat1 are identity
passthroughs (the attention computed for the first two layers is
discarded).  Only the last layer does work:

    sam_sub = sam_masks[:, 1:, ::16, ::16]           # nearest-resize to 32x32
    win(p)  = 1 + argmax-style last mask covering pixel p (0 = background)
    tm[m,p] = onehot                                  # [32, 1024] per batch
    mf      = l2norm( (tm @ feat2_flat^T) / (cnt + 1e-5) )
    q       = mf @ W2.T + b2
    cos     = (q q^T) / (|q_i||q_j| + 1e-8)
    gp      = l2norm( diag(1/(rowsum(cos)+1e-8)) cos @ q )
    mf_f    = mf + sigmoid(g2) * gp
    out2    = feat2 + reshape(mf_f^T @ tm)

Sharding: 8 cores = 4 batches x 2 channel-halves.  Each core reads the
full (channel-rolled) feature map of its batch to compute the full
pooled/attended mask features redundantly (no collectives), then emits
out2 for its 512-channel half.  The channel roll makes the SPMD program
identical across cores: every core's own channels are [0:512] in its
rolled coordinate system.

The pooling contraction needs feat^T ([pixel, channel]); that layout is
produced with the DMA x-bar transpose, which only supports 2-byte
dtypes, so a bf16 copy of feat2 is staged host-side.  bf16 is only used
for matmul operands whose error is attenuated (pool sums, W projection);
the residual add out2 = feat2 + fs stays fp32.  Validated end-to-end
rel. error ~1e-4.
"""

import os
import tempfile

import numpy as np


def _install_trace_hook():
    """Bridge trn_agent_boot's NTFF profiling into antenv.axon_hooks
    (dev-only path, used when KERNEL_TRACE=1)."""
    import sys, types
    try:
        import antenv.axon_hooks  # noqa: F401
        return
    except ImportError:
        pass
    from trn_agent_boot.trn_boot import _ntff_profile_via_ctypes
    hook = _ntff_profile_via_ctypes("/opt/axon/libaxon_pjrt.so")
    mod = types.ModuleType("antenv.axon_hooks")
    mod.get_axon_ntff_profile_hook = lambda: hook
    mod.set_axon_ntff_profile_hook = lambda h: None
    sys.modules["antenv.axon_hooks"] = mod
    import antenv
    antenv.axon_hooks = mod
    from concourse import bass_utils
    bass_utils.upload_artifacts = lambda tmpdir: f"file://{tmpdir}"


B = 4
M = 32        # masks after dropping the first
C = 1024      # feat2 channels
P = 1024      # 32*32 pixels
CH = 512      # channels per core
N_CORES = 8

_CACHE = {}


def _build_program():
    import concourse.bass as bass
    import concourse.tile as tile
    from concourse import mybir, bacc
    from concourse.masks import make_identity

    f32 = mybir.dt.float32
    bf16 = mybir.dt.bfloat16
    i32 = mybir.dt.int32
    Alu = mybir.AluOpType
    Act = mybir.ActivationFunctionType

    nc = bacc.Bacc("TRN2", target_bir_lowering=False, enable_partition_id=False)

    featTd = nc.dram_tensor("featT", [P, C], bf16, kind="ExternalInput")
    feath = nc.dram_tensor("feath", [CH, P], f32, kind="ExternalInput")
    wt = nc.dram_tensor("wt", [C, C], bf16, kind="ExternalInput")
    sam = nc.dram_tensor("sam", [P, M], i32, kind="ExternalInput")
    bias = nc.dram_tensor("bias", [1, C], f32, kind="ExternalInput")
    sig = nc.dram_tensor("sig", [1], f32, kind="ExternalInput")
    out = nc.dram_tensor("out", [CH, P], f32, kind="ExternalOutput")

    with tile.TileContext(nc) as tc:
      with tc.tile_pool(name="persist", bufs=1) as persist:
        def _t(tc, shape, dtype, name):
            return persist.tile(shape, dtype, tag=name, name=name)

        # ---- constants ------------------------------------------------
        ident_f = _t(tc, [128, 128], f32, "ident_f")
        make_identity(nc, ident_f)
        ident_b = _t(tc, [128, 128], bf16, "ident_b")
        make_identity(nc, ident_b)
        ones_b = _t(tc, [128, 1], bf16, "ones_b")
        nc.vector.memset(ones_b[:], 1.0)
        iota1 = _t(tc, [128, 8, M], i32, "iota1")
        nc.gpsimd.iota(iota1[:], pattern=[[0, 8], [1, M]], base=1,
                       channel_multiplier=0)

        bias_bc = _t(tc, [M, C], f32, "bias_bc")
        bias_ap = bias[:]
        nc.gpsimd.dma_start(
            out=bias_bc[:],
            in_=bass.AP(tensor=bias_ap.tensor, offset=bias_ap.offset,
                        ap=[[0, M]] + list(bias_ap.ap[1:])))
        sig_t = _t(tc, [M, 1], f32, "sig_t")
        sig_ap = sig[:]
        nc.gpsimd.dma_start(
            out=sig_t[:],
            in_=bass.AP(tensor=sig_ap.tensor, offset=sig_ap.offset,
                        ap=[[0, M]] + list(sig_ap.ap)))

        # ---- big loads ------------------------------------------------
        # feat^T is staged host-side (bf16, [pixel, channel]) so no DMA
        # x-bar transposes are needed; loads spread over HWDGE + SWDGE.
        sam_all = _t(tc, [128, 8, M], i32, "sam_all")
        sam_r = sam.rearrange("(j p) m -> p j m", p=128)
        nc.sync.dma_start(out=sam_all[:], in_=sam_r)

        # Loads are phase-sequenced (featT -> wt -> feath) with explicit
        # deps so the earlier-needed tensors get the full HBM bandwidth
        # instead of fair-sharing with loads only needed much later.
        featT = _t(tc, [128, 8, C], bf16, "featT")   # [pix, ptile, ch]
        featT_r = featTd.rearrange("(j p) c -> p j c", p=128)
        ft_dmas = []
        for j in range(8):
            eng = nc.sync if j % 2 == 0 else nc.scalar
            ft_dmas.append(
                eng.dma_start(out=featT[:, j, :], in_=featT_r[:, j, :]))

        wt_all = _t(tc, [128, 8, C], bf16, "wt_all")  # [cin, ktile, cout]
        wt_dmas = []
        for k in range(8):
            eng = nc.sync if k % 2 == 0 else nc.scalar
            wt_dmas.append(
                eng.dma_start(out=wt_all[:, k, :],
                              in_=wt[k * 128:(k + 1) * 128, :]))

        # feath is only needed by the final residual add; gate it behind
        # wt (and keep it on sync, which runs no compute) so featT/wt get
        # the HBM bandwidth first.
        feath_all = _t(tc, [128, 4, P], f32, "feath_all")
        for i in range(4):
            fh = nc.sync.dma_start(out=feath_all[:, i, :],
                                   in_=feath[i * 128:(i + 1) * 128, :])
            for dep in wt_dmas[6:]:
                tile.add_dep_helper(fh.ins, dep.ins, reason="feath after wt")

        with (
            tc.tile_pool(name="work", bufs=2) as work,
            tc.tile_pool(name="opool", bufs=4) as opool,
            tc.tile_pool(name="psA", bufs=1, space=bass.MemorySpace.PSUM) as psA,
            tc.tile_pool(name="psQ", bufs=1, space=bass.MemorySpace.PSUM) as psQ,
            tc.tile_pool(name="psT", bufs=2, space=bass.MemorySpace.PSUM) as psT,
            tc.tile_pool(name="psF", bufs=2, space=bass.MemorySpace.PSUM) as psF,
        ):
            # PSUM budget (8 banks): psA tag "acc" = 2 banks (ring of 1:
            # mf -> q -> simq), psT tag "sm" = 2 banks (ring of 2, all
            # small tiles), psF tag "fs" = 4 banks (ring of 2, shared by
            # the tm transposes and the fs accumulators).
            ones_col = _t(tc, [M, 1], bf16, "ones_col")
            nc.vector.memset(ones_col[:], 1.0)

            # preload the ACT function table used by Sqrt/Square off the
            # critical path
            one_f = work.tile([1, 1], f32, tag="one_f")
            nc.vector.memset(one_f[:], 1.0)
            warm_act = work.tile([1, 1], f32, tag="warm_act")
            nc.scalar.sqrt(warm_act[:], one_f[:])

            # Ungated junk matmuls dropped into the PE stream at phase
            # boundaries: they fill PE-idle windows (keeping the HAM
            # clock-gate at 2.4 GHz) and never gate real work.
            def _warm(tag, n=4, rhs=None, k=128):
                if rhs is None:
                    rhs = featT[:, 0, 0:512]
                wps = psF.tile([128, 512], f32, tag="fs", name=f"w_{tag}")
                nfree = rhs.free_size()
                for w in range(n):
                    nc.tensor.matmul(wps[:, 0:nfree], lhsT=ident_b[0:k, :],
                                     rhs=rhs, start=(w == 0), stop=(w == n - 1))
                sink = work.tile([1, 1], f32, tag="sink", name=f"s_{tag}")
                nc.vector.tensor_copy(sink[:], wps[0:1, 0:1])

            # ---- one-hot masks (tm) ----------------------------------
            t1 = work.tile([128, 8, M], i32, tag="t1")
            nc.vector.tensor_tensor(out=t1[:], in0=sam_all[:], in1=iota1[:],
                                    op=Alu.mult)
            win = work.tile([128, 8], i32, tag="win")
            nc.vector.reduce_max(out=win[:], in_=t1[:],
                                 axis=mybir.AxisListType.X)
            win_f = work.tile([128, 8], f32, tag="win_f")
            nc.vector.tensor_copy(win_f[:], win[:])
            iota_f = _t(tc, [128, 8, M], f32, "iota_f")
            nc.vector.tensor_copy(iota_f[:], iota1[:])
            tm_all = _t(tc, [128, 8, M], bf16, "tm_all")
            for j in range(8):
                nc.vector.tensor_scalar(
                    out=tm_all[:, j, :], in0=iota_f[:, j, :],
                    scalar1=win_f[:, j:j + 1], scalar2=None, op0=Alu.is_equal)

            # ---- tm^T ([mask, pixel]) + counts, early ----------------
            tm_mp = _t(tc, [M, P], bf16, "tm_mp")
            tt_ps = psT.tile([M, P], bf16, tag="sm")
            for j in range(8):
                nc.tensor.transpose(tt_ps[:, j * 128:(j + 1) * 128],
                                    tm_all[:, j, :], ident_b[:])
            nc.vector.tensor_copy(tm_mp[:], tt_ps[:])
            cnt_s = work.tile([M, 1], f32, tag="cnt_s")
            nc.vector.reduce_sum(out=cnt_s[:], in_=tm_mp[:],
                                 axis=mybir.AxisListType.X)

            _warm("w0")

            # ---- masked pooling: mf = tm^T @ featT -------------------
            mf_ps = psA.tile([M, C], f32, tag="acc")
            for j in range(8):
                st, sp = (j == 0), (j == 7)
                nc.tensor.matmul(mf_ps[:, 0:512], lhsT=tm_all[:, j, :],
                                 rhs=featT[:, j, 0:512], start=st, stop=sp)
                nc.tensor.matmul(mf_ps[:, 512:1024], lhsT=tm_all[:, j, :],
                                 rhs=featT[:, j, 512:1024], start=st, stop=sp)

            # ---- raw-mf^T (bf16): emitted first so the PSUM-reader
            # serialization doesn't chain the q path behind the norm chain
            mf_sb = _t(tc, [M, C], bf16, "mf_sb")
            nc.scalar.copy(mf_sb[:], mf_ps[:])
            mfnT = _t(tc, [128, 8, M], bf16, "mfnT")
            tp_ps = psT.tile([128, 8, M], bf16, tag="sm")
            for j in range(8):
                nc.tensor.transpose(tp_ps[:, j, :],
                                    mf_sb[:, j * 128:(j + 1) * 128],
                                    ident_b[0:M, 0:M])
            nc.scalar.copy(mfnT[:], tp_ps[:])

            # ---- normalize: mf_n = l2norm(mf / (cnt + 1e-5)) ---------
            nc.vector.tensor_scalar_add(cnt_s[:], cnt_s[:], 1e-5)
            inv_c = work.tile([M, 1], f32, tag="inv_c")
            nc.vector.reciprocal(inv_c[:], cnt_s[:])
            sq_scr = _t(tc, [M, C], f32, "sq_scr")
            n2p = work.tile([M, 1], f32, tag="n2p")
            nc.scalar.activation(sq_scr[:], mf_ps[:], Act.Square,
                                 accum_out=n2p[:])
            npool = work.tile([M, 1], f32, tag="npool")
            nc.scalar.sqrt(npool[:], n2p[:])
            den = work.tile([M, 1], f32, tag="den")
            nc.vector.tensor_scalar(out=den[:], in0=npool[:],
                                    scalar1=inv_c[:, 0:1], scalar2=1e-12,
                                    op0=Alu.mult, op1=Alu.max)
            invd = work.tile([M, 1], f32, tag="invd")
            nc.vector.reciprocal(invd[:], den[:])
            scale_mf = work.tile([M, 1], f32, tag="scale_mf")
            nc.vector.tensor_mul(scale_mf[:], inv_c[:], invd[:])
            mf_n = _t(tc, [M, C], f32, "mf_n")
            nc.vector.tensor_scalar_mul(mf_n[:], mf_ps[:], scale_mf[:, 0:1])

            _warm("wa")

            # ---- q_raw = mf @ W^T; q = scale*q_raw + b afterwards ----
            # (the l2norm row-scale commutes past W^T, so the projection
            # overlaps the norm chain instead of waiting for it)
            q_ps = psQ.tile([M, C], f32, tag="q")
            for k in range(8):
                st, sp = (k == 0), (k == 7)
                nc.tensor.matmul(q_ps[:, 0:512], lhsT=mfnT[:, k, :],
                                 rhs=wt_all[:, k, 0:512], start=st, stop=sp)
                nc.tensor.matmul(q_ps[:, 512:1024], lhsT=mfnT[:, k, :],
                                 rhs=wt_all[:, k, 512:1024], start=st,
                                 stop=sp)

            _warm("wb")

            q_b = _t(tc, [M, C], bf16, "q_b")
            nc.vector.scalar_tensor_tensor(
                out=q_b[:], in0=q_ps[:], scalar=scale_mf[:, 0:1],
                in1=bias_bc[:], op0=Alu.mult, op1=Alu.add)
            sq_scr2 = _t(tc, [M, C], f32, "sq_scr2")
            n2q = work.tile([M, 1], f32, tag="n2q")
            nc.scalar.activation(sq_scr2[:], q_b[:], Act.Square,
                                 accum_out=n2q[:])
            n_q = work.tile([M, 1], f32, tag="n_q")
            nc.scalar.sqrt(n_q[:], n2q[:])

            # ---- cos = (q q^T) / (outer(n, n) + 1e-8) ----------------
            nps = psT.tile([1, M], f32, tag="sm")
            nc.tensor.transpose(nps[:], n_q[:], ident_f[0:M, 0:M])
            n_row = work.tile([1, M], f32, tag="n_row")
            nc.scalar.copy(n_row[:], nps[:])
            outer_ps = psT.tile([M, M], f32, tag="sm")
            nc.tensor.matmul(outer_ps[:], lhsT=n_row[:], rhs=n_row[:],
                             start=True, stop=True)
            den2 = work.tile([M, M], f32, tag="den2")
            nc.vector.tensor_scalar_add(den2[:], outer_ps[:], 1e-8)
            nc.vector.reciprocal(den2[:], den2[:])

            qT = _t(tc, [128, 8, M], bf16, "qT")
            tq_ps = psT.tile([128, 8, M], bf16, tag="sm")
            for j in range(8):
                nc.tensor.transpose(tq_ps[:, j, :],
                                    q_b[:, j * 128:(j + 1) * 128],
                                    ident_b[0:M, 0:M])
            nc.vector.tensor_copy(qT[:], tq_ps[:])

            dot_ps = psT.tile([M, M], f32, tag="sm")
            for j in range(8):
                nc.tensor.matmul(dot_ps[:], lhsT=qT[:, j, :], rhs=qT[:, j, :],
                                 start=(j == 0), stop=(j == 7))
            _warm("wc", rhs=q_b[:, 0:512], k=M)
            cos_b = work.tile([M, M], bf16, tag="cos_b")
            nc.vector.tensor_mul(cos_b[:], dot_ps[:], den2[:])

            # ---- simq = cos @ q ; rowsum = cos @ 1 -------------------
            sq_ps = psQ.tile([M, C], f32, tag="q")
            for s, e in ((0, 512), (512, 1024)):
                nc.tensor.matmul(sq_ps[:, s:e], lhsT=cos_b[:], rhs=q_b[:, s:e],
                                 start=True, stop=True)
            rs_ps = psT.tile([M, 1], f32, tag="sm")
            nc.tensor.matmul(rs_ps[:], lhsT=cos_b[:], rhs=ones_col[:],
                             start=True, stop=True)
            _warm("wd", rhs=qT[:, 0:8, :], k=128)

            # mf_f = mf_n + coef * simq, with
            # coef = sig * rs / max(|rs| * ||simq_row||, 1e-12),
            # rs = 1/(rowsum + 1e-8)   (== sig * l2norm of the gp row)
            rs = work.tile([M, 1], f32, tag="rs")
            nc.vector.tensor_scalar_add(rs[:], rs_ps[:], 1e-8)
            nc.vector.reciprocal(rs[:], rs[:])
            sq_scr3 = _t(tc, [M, C], f32, "sq_scr3")
            n2g = work.tile([M, 1], f32, tag="n2g")
            nc.scalar.activation(sq_scr3[:], sq_ps[:], Act.Square,
                                 accum_out=n2g[:])
            nsq = work.tile([M, 1], f32, tag="nsq")
            nc.scalar.sqrt(nsq[:], n2g[:])
            rneg = work.tile([M, 1], f32, tag="rneg")
            nc.vector.tensor_scalar_mul(rneg[:], rs[:], -1.0)
            rabs = work.tile([M, 1], f32, tag="rabs")
            nc.vector.tensor_max(rabs[:], rs[:], rneg[:])
            den3 = work.tile([M, 1], f32, tag="den3")
            nc.vector.tensor_mul(den3[:], rabs[:], nsq[:])
            nc.vector.tensor_scalar_max(den3[:], den3[:], 1e-12)
            inv3 = work.tile([M, 1], f32, tag="inv3")
            nc.vector.reciprocal(inv3[:], den3[:])
            coef = work.tile([M, 1], f32, tag="coef")
            nc.vector.tensor_mul(coef[:], rs[:], inv3[:])
            nc.vector.tensor_mul(coef[:], coef[:], sig_t[:])

            mf_fb = _t(tc, [M, CH], bf16, "mf_fb")
            nc.vector.scalar_tensor_tensor(
                out=mf_fb[:], in0=sq_ps[:, 0:CH], scalar=coef[:, 0:1],
                in1=mf_n[:, 0:CH], op0=Alu.mult, op1=Alu.add)

            # ---- fs = mf_f^T @ tm ; out = feat + fs ------------------
            # The residual adds are spread over DVE (direct from PSUM)
            # and ACT-copy + GpSimd-add so no single engine serializes
            # the tail; stores split across the two HWDGE queues.
            for i in range(4):
                for hx, (s, e) in enumerate(((0, 512), (512, 1024))):
                    u = i * 2 + hx
                    fs_ps = psF.tile([128, 512], f32, tag="fs")
                    nc.tensor.matmul(fs_ps[:],
                                     lhsT=mf_fb[:, i * 128:(i + 1) * 128],
                                     rhs=tm_mp[:, s:e], start=True, stop=True)
                    o_t = opool.tile([128, 512], f32, tag="o_t")
                    if u not in (1, 4):
                        nc.vector.tensor_add(o_t[:], fs_ps[:],
                                             feath_all[:, i, s:e])
                    else:
                        fs_sb = opool.tile([128, 512], f32, tag="fs_sb")
                        nc.scalar.copy(fs_sb[:], fs_ps[:])
                        nc.gpsimd.tensor_tensor(out=o_t[:], in0=fs_sb[:],
                                                in1=feath_all[:, i, s:e],
                                                op=Alu.add)
                    eng = nc.sync if u % 2 == 0 else nc.scalar  # split stores
                    eng.dma_start(out=out[i * 128:(i + 1) * 128, s:e],
                                  in_=o_t[:])

    nc.compile()
    return nc


def _get_program():
    if "nc" not in _CACHE:
        _CACHE["nc"] = _build_program()
    return _CACHE["nc"]


def kernel(feat0, feat1, feat2, sam_masks, W0, b0, W1, b1, W2, b2, g0, g1, g2):
    import ml_dtypes
    from concourse.bass_utils import run_bass_kernel_spmd

    bf16 = ml_dtypes.bfloat16
    feat0 = np.asarray(feat0, np.float32)
    feat1 = np.asarray(feat1, np.float32)
    feat2 = np.asarray(feat2, np.float32)
    sam_masks = np.asarray(sam_masks, np.int32)
    W2 = np.asarray(W2, np.float32)
    b2 = np.asarray(b2, np.float32)
    g2 = np.float32(np.asarray(g2))

    nc = _get_program()

    feat2_r = feat2.reshape(B, C, P)
    fT = np.ascontiguousarray(feat2_r.transpose(0, 2, 1)).astype(bf16)
    fT_roll = np.ascontiguousarray(
        np.concatenate([fT[:, :, CH:], fT[:, :, :CH]], axis=2))  # h=1 variant

    WT = np.ascontiguousarray(W2.T).astype(bf16)  # [cin, cout]
    WT_roll = np.ascontiguousarray(
        np.roll(np.roll(WT, -CH, axis=0), -CH, axis=1))
    b_roll = np.roll(b2, -CH)
    b2_bf = np.ascontiguousarray(b2.reshape(1, C))
    b_roll_bf = np.ascontiguousarray(b_roll.reshape(1, C))
    sig = np.float32(1.0) / (np.float32(1.0) + np.exp(-g2, dtype=np.float32))
    sig = np.asarray([sig], np.float32)

    # [B, p, m] int32, pixels at stride 16 of the 512-grid
    sam_pm = np.ascontiguousarray(
        sam_masks[:, 1:, ::16, ::16].reshape(B, M, P).transpose(0, 2, 1))

    in_maps = []
    for core in range(N_CORES):
        b, h = divmod(core, 2)
        in_maps.append({
            "featT": fT[b] if h == 0 else fT_roll[b],
            "feath": np.ascontiguousarray(feat2_r[b, h * CH:(h + 1) * CH]),
            "wt": WT if h == 0 else WT_roll,
            "sam": sam_pm[b],
            "bias": (b2_bf if h == 0 else b_roll_bf),
            "sig": sig,
        })

    trace = bool(int(os.environ.get("KERNEL_TRACE", "0")))
    kw = {}
    if trace:
        _install_trace_hook()
        kw = dict(trace=True, tmpdir=tempfile.mkdtemp(prefix="kernel_trace_"))
        tc_env = os.environ.get("KERNEL_TRACE_CORES")
        if tc_env:
            kw["trace_cores"] = [int(x) for x in tc_env.split(",")]
    res = run_bass_kernel_spmd(nc, in_maps, core_ids=list(range(N_CORES)), **kw)
    _CACHE["last_result"] = res

    out2 = np.empty((B, C, P), np.float32)
    for core in range(N_CORES):
        b, h = divmod(core, 2)
        out2[b, h * CH:(h + 1) * CH] = res.results[core]["out"]

    return feat0, feat1, out2.reshape(B, C, 32, 32)
